# revision 1
# baseline (speedup 1.0000x reference)
"""Trainium2 Bass kernel for nn_CrossAttentionFusion (cross-attention + BitLinear FFN).

Sharding: 8 cores = 4 batches x 2 sequence-halves. Each core:
  - owns 1024 query tokens (sem shard, feature-major),
  - computes K/V for its batch's full 2048 tokens from pro (feature-major),
  - runs full attention for its queries + BitLinear FFN, writes its out^T shard.
No collectives needed; host does all layout transposes and the final gather.
"""
import math
import numpy as np
from contextlib import ExitStack

import concourse.bass as bass
import concourse.bass_isa as bass_isa
import concourse.tile as tile
from concourse import bacc, mybir
from concourse.bass_utils import run_bass_kernel_spmd

F32 = mybir.dt.float32
BF16 = mybir.dt.bfloat16
FP8 = mybir.dt.float8e4
AF = mybir.ActivationFunctionType
ALU = mybir.AluOpType

B, S, DS, DP, H = 4, 2048, 1024, 512, 8
DF = 4 * DS
HD = DS // H          # 128
TOK = 1024            # query tokens per core
N_CORES = 8
EPS = 1e-6
C_RND = 12582912.0    # 1.5 * 2**23 : +C-C = round-to-nearest-even
QK_SCALE = 1.0 / math.sqrt(HD)

P = 128
M_SEM = DS // P       # 8
M_PRO = DP // P       # 4
M_FF = DF // P        # 32
NT_Q = TOK // 512     # 2
NT_K = S // P         # 16
MT_V = S // P         # 16


def bcast_free(ap2d, rep):
    """[P, W] AP -> [P, rep, W] AP with step-0 middle dim (free broadcast)."""
    return bass.AP(tensor=ap2d.tensor, offset=ap2d.offset,
                   ap=[ap2d.ap[0], [0, rep], ap2d.ap[1]])


def build_nc(debug_outs=False):
    nc = bacc.Bacc("TRN2", target_bir_lowering=False, debug=False, num_devices=N_CORES)

    semT = nc.dram_tensor("semT", [DS, TOK], F32, kind="ExternalInput").ap()
    proT = nc.dram_tensor("proT", [DP, S], F32, kind="ExternalInput").ap()
    wqT = nc.dram_tensor("wqT", [DS, DS], BF16, kind="ExternalInput").ap()
    wkT = nc.dram_tensor("wkT", [DP, DS], BF16, kind="ExternalInput").ap()
    wvT = nc.dram_tensor("wvT", [DP, DS], BF16, kind="ExternalInput").ap()
    woT = nc.dram_tensor("woT", [DS, DS], BF16, kind="ExternalInput").ap()
    w1T = nc.dram_tensor("w1T", [DS, DF], F32, kind="ExternalInput").ap()
    w2T = nc.dram_tensor("w2T", [DF, DS], F32, kind="ExternalInput").ap()
    w1s = nc.dram_tensor("w1s", [P, DF], F32, kind="ExternalInput").ap()
    w2s = nc.dram_tensor("w2s", [DP, DS], F32, kind="ExternalInput").ap()
    gsem = nc.dram_tensor("gsem", [P, M_SEM], F32, kind="ExternalInput").ap()
    gpro = nc.dram_tensor("gpro", [P, M_PRO], F32, kind="ExternalInput").ap()
    gff = nc.dram_tensor("gff", [P, M_SEM], F32, kind="ExternalInput").ap()
    bq = nc.dram_tensor("bq", [P, M_SEM], F32, kind="ExternalInput").ap()
    bk = nc.dram_tensor("bk", [P, M_SEM], F32, kind="ExternalInput").ap()
    bv = nc.dram_tensor("bv", [P, M_SEM], F32, kind="ExternalInput").ap()
    bo = nc.dram_tensor("bo", [P, M_SEM], F32, kind="ExternalInput").ap()
    alpha = nc.dram_tensor("alpha", [P, M_FF], F32, kind="ExternalInput").ap()
    beta = nc.dram_tensor("beta", [P, M_FF], F32, kind="ExternalInput").ap()
    outT = nc.dram_tensor("outT", [DS, TOK], F32, kind="ExternalOutput").ap()

    dbg = {}
    if debug_outs:
        for name, shape, dt in [
            ("dbg_semn", [DS, TOK], BF16), ("dbg_q", [DS, TOK], BF16),
            ("dbg_k", [DS, S], BF16), ("dbg_v", [S, DS], BF16),
            ("dbg_ctx", [DS, TOK], BF16), ("dbg_semout", [DS, TOK], F32),
            ("dbg_xq", [DS, TOK], BF16), ("dbg_h", [DF, TOK], BF16),
            ("dbg_hq", [DF, TOK], BF16), ("dbg_mw", [1, 2], F32),
        ]:
            dbg[name] = nc.dram_tensor(name, shape, dt, kind="ExternalOutput").ap()

    with tile.TileContext(nc) as tc, ExitStack() as top:
        persist = top.enter_context(tc.tile_pool(name="persist", bufs=1))
        rows = top.enter_context(tc.tile_pool(name="rows", bufs=1))
        ps_mm = top.enter_context(tc.tile_pool(name="ps_mm", bufs=2, space="PSUM"))
        pdram_w = top.enter_context(tc.tile_pool(name="pdram_w", bufs=1,
                                                 space="DRAM"))
        w1q_d = pdram_w.tile([P, M_FF, M_SEM, P], BF16)
        w2q_d = pdram_w.tile([P, M_SEM, M_FF, P], BF16)

        ones = persist.tile([P, 1], BF16)
        nc.vector.memset(ones[:], 1.0)
        ones_row = persist.tile([1, P], BF16)
        nc.vector.memset(ones_row[:], 1.0)
        eps_t = persist.tile([1, 1], F32)
        nc.vector.memset(eps_t[:], EPS)

        gsem_sb = persist.tile([P, M_SEM], F32)
        gpro_sb = persist.tile([P, M_PRO], F32)
        gff_sb = persist.tile([P, M_SEM], F32)
        bq_sb = persist.tile([P, M_SEM], F32)
        bk_sb = persist.tile([P, M_SEM], F32)
        bv_sb = persist.tile([P, M_SEM], F32)
        bo_sb = persist.tile([P, M_SEM], F32)
        alpha_sb = persist.tile([P, M_FF], F32)
        rbeta_sb = persist.tile([P, M_FF], F32)
        for ap_d, t in [(gsem, gsem_sb), (gpro, gpro_sb), (gff, gff_sb),
                        (bq, bq_sb), (bk, bk_sb), (bv, bv_sb), (bo, bo_sb),
                        (alpha, alpha_sb)]:
            nc.sync.dma_start(t[:], ap_d[:])
        beta_t = persist.tile([P, M_FF], F32)
        nc.sync.dma_start(beta_t[:], beta[:])
        nc.vector.tensor_scalar(rbeta_sb[:], beta_t[:], 1e-9, None, ALU.add)
        nc.vector.reciprocal(rbeta_sb[:], rbeta_sb[:])

        semT_r = semT.rearrange("(m p) t -> p m t", p=P)

        def rmsnorm_fm(pool, fetch, nm, T, g_sb, out_bf):
            """feature-major rmsnorm: out_bf[:, m, :] = x_m * g_m * rsqrt(ms+eps)"""
            D = nm * P
            rs_row = pool.tile([1, T], F32, tag="rs_row", bufs=1)
            xs = [fetch(m) for m in range(nm)]
            for ch in range(T // 512):
                pst = ps_mm.tile([P, 512], F32, tag="mm")
                ps = pst[0:1, :]
                for m in range(nm):
                    sq = pool.tile([P, 512], BF16, tag="sq", bufs=3)
                    nc.scalar.activation(sq[:], xs[m][:, ch * 512:(ch + 1) * 512],
                                         AF.Square)
                    nc.tensor.matmul(ps[:], ones[:], sq[:],
                                     start=(m == 0), stop=(m == nm - 1))
                nc.scalar.activation(rs_row[:, ch * 512:(ch + 1) * 512], ps[:],
                                     AF.Ln, bias=eps_t[:], scale=1.0 / D)
            nc.scalar.activation(rs_row[:], rs_row[:], AF.Exp, scale=-0.5)
            rs_bc = pool.tile([P, T], F32, tag="rs_bc", bufs=1)
            nc.gpsimd.partition_broadcast(rs_bc[:], rs_row[:])
            for m in range(nm):
                nc.vector.scalar_tensor_tensor(
                    out=out_bf[:, m, :], in0=xs[m][:],
                    scalar=g_sb[:, m:m + 1], in1=rs_bc[:],
                    op0=ALU.mult, op1=ALU.mult)

        # ================= phase 1: input norms =================
        es_norm = ExitStack()
        pnorm = es_norm.enter_context(tc.tile_pool(name="pnorm", bufs=1))
        semn_sb = pnorm.tile([P, M_SEM, TOK], BF16)
        pron_sb = pnorm.tile([P, M_PRO, S], BF16)

        with tc.tile_pool(name="pin1", bufs=1) as pin1:
            semT_sb = pin1.tile([P, M_SEM, TOK], F32)
            nc.sync.dma_start(semT_sb[:], semT_r)
            rmsnorm_fm(pin1, lambda m: semT_sb[:, m, :], M_SEM, TOK, gsem_sb, semn_sb)

        with tc.tile_pool(name="pin2", bufs=1, side="right") as pin2:
            proT_sb = pin2.tile([P, M_PRO, S], F32)
            nc.sync.dma_start(proT_sb[:], proT.rearrange("(m p) t -> p m t", p=P))
            rmsnorm_fm(pin2, lambda m: proT_sb[:, m, :], M_PRO, S, gpro_sb, pron_sb)

        if debug_outs:
            nc.sync.dma_start(dbg["dbg_semn"].rearrange("(m p) t -> p m t", p=P),
                              semn_sb[:])

        # ================= phase 3: Q/K/V =================
        es_qkv = ExitStack()
        pqkv = es_qkv.enter_context(tc.tile_pool(name="pqkv", bufs=1, side="right"))
        q_sb = pqkv.tile([P, M_SEM, TOK], FP8)
        k_sb = pqkv.tile([P, M_SEM, S], FP8)
        v_sb = pqkv.tile([P, MT_V, DS], BF16)

        with tc.tile_pool(name="pw3", bufs=1) as pw3:
            wq_sb = pw3.tile([P, M_SEM, DS], BF16)
            nc.sync.dma_start(wq_sb[:], wqT.rearrange("(m p) o -> p m o", p=P))
            for m in range(M_SEM):
                for n in range(NT_Q):
                    ps = ps_mm.tile([P, 512], F32, tag="mm")
                    for kk in range(M_SEM):
                        nc.tensor.matmul(ps[:], wq_sb[:, kk, m * P:(m + 1) * P],
                                         semn_sb[:, kk, n * 512:(n + 1) * 512],
                                         start=(kk == 0), stop=(kk == M_SEM - 1))
                    nc.scalar.activation(q_sb[:, m, n * 512:(n + 1) * 512], ps[:],
                                         AF.Identity, bias=bq_sb[:, m:m + 1])

            wk_sb = pw3.tile([P, M_PRO, DS], BF16)
            nc.sync.dma_start(wk_sb[:], wkT.rearrange("(m p) o -> p m o", p=P))
            for m in range(M_SEM):
                for n in range(S // 512):
                    ps = ps_mm.tile([P, 512], F32, tag="mm")
                    for kk in range(M_PRO):
                        nc.tensor.matmul(ps[:], wk_sb[:, kk, m * P:(m + 1) * P],
                                         pron_sb[:, kk, n * 512:(n + 1) * 512],
                                         start=(kk == 0), stop=(kk == M_PRO - 1))
                    nc.scalar.activation(k_sb[:, m, n * 512:(n + 1) * 512], ps[:],
                                         AF.Identity, bias=bk_sb[:, m:m + 1])

            wv_sb = pw3.tile([P, M_PRO, DS], BF16)
            nc.sync.dma_start(wv_sb[:], wvT.rearrange("(m p) o -> p m o", p=P))
            for mt in range(MT_V):
                for n in range(DS // 512):
                    ps = ps_mm.tile([P, 512], F32, tag="mm")
                    for kk in range(M_PRO):
                        nc.tensor.matmul(ps[:], pron_sb[:, kk, mt * P:(mt + 1) * P],
                                         wv_sb[:, kk, n * 512:(n + 1) * 512],
                                         start=(kk == 0), stop=(kk == M_PRO - 1))
                    # bias bv folded in at ctx evac
                    nc.scalar.activation(v_sb[:, mt, n * 512:(n + 1) * 512], ps[:],
                                         AF.Copy)
        es_norm.close()   # semn/pron freed

        # ===== phase 2: mean(|w|) via per-core strips + AllReduce =====
        with tc.tile_pool(name="pwmean", bufs=2) as pwm, \
             tc.tile_pool(name="pdram", bufs=1, space="DRAM") as pdram:
            def strip_sum(ws_ap, nrows, cols, name):
                ntile = nrows // P
                nch = cols // 1024
                mcols = rows.tile([P, ntile * nch], F32, tag=f"mcols_{name}")
                for j in range(ntile):
                    for ci in range(nch):
                        wt = pwm.tile([P, 1024], F32, tag="wmean")
                        nc.sync.dma_start(
                            wt[:], ws_ap[j * P:(j + 1) * P,
                                         ci * 1024:(ci + 1) * 1024])
                        nc.scalar.activation(wt[:], wt[:], AF.Abs,
                                             accum_out=mcols[:, j * nch + ci:
                                                             j * nch + ci + 1])
                msum = rows.tile([P, 1], F32, tag=f"msum_{name}")
                nc.vector.tensor_reduce(msum[:], mcols[:], axis=mybir.AxisListType.X,
                                        op=ALU.add)
                msum_all = rows.tile([P, 1], F32, tag=f"msuma_{name}")
                nc.gpsimd.partition_all_reduce(msum_all[:], msum[:], P,
                                               bass_isa.ReduceOp.add)
                return msum_all

            s1 = strip_sum(w1s, P, DF, "w1")
            s2 = strip_sum(w2s, DP, DS, "w2")
            loc = rows.tile([1, 2], F32, tag="ccloc")
            nc.vector.tensor_copy(loc[:, 0:1], s1[0:1, :])
            nc.vector.tensor_copy(loc[:, 1:2], s2[0:1, :])
            cin = pdram.tile([1, 2], F32)
            cout = pdram.tile([1, 2], F32)
            nc.sync.dma_start(cin[:], loc[:])
            nc.gpsimd.collective_compute(
                "AllReduce", ALU.add,
                replica_groups=[list(range(N_CORES))],
                ins=[cin.opt()], outs=[cout.opt()])
            tot = rows.tile([1, 2], F32, tag="cctot")
            nc.sync.dma_start(tot[:], cout[:])
            mwrow = rows.tile([1, 2], F32, tag="mwrow")
            nc.vector.tensor_scalar(mwrow[:, 0:1], tot[:, 0:1], 1.0 / (DS * DF),
                                    None, ALU.mult)
            nc.vector.tensor_scalar(mwrow[:, 1:2], tot[:, 1:2], 1.0 / (DF * DS),
                                    None, ALU.mult)
            mw_all = rows.tile([P, 2], F32, tag="mwall")
            nc.gpsimd.partition_broadcast(mw_all[:], mwrow[:])
            mw1, mw2 = mw_all[:, 0:1], mw_all[:, 1:2]
            sw_all = rows.tile([P, 2], F32, tag="swall")
            nc.vector.reciprocal(sw_all[:], mw_all[:])
            sw1_bc, sw2_bc = sw_all[:, 0:1], sw_all[:, 1:2]
        if debug_outs:
            nc.sync.dma_start(dbg["dbg_mw"][:], mwrow[:])

        # folded snake scalars: alphap = alpha*mw1 ; rbetap = rbeta/mw1
        alphap = persist.tile([P, M_FF], F32)
        rbetap = persist.tile([P, M_FF], F32)
        nc.vector.tensor_scalar(alphap[:], alpha_sb[:], mw1, None, ALU.mult)
        nc.vector.tensor_scalar(rbetap[:], rbeta_sb[:], sw1_bc, None, ALU.mult)

        # pre-ternarize W1/W2 into DRAM (overlaps QKV/attention)
        with tc.tile_pool(name="ptern", bufs=1, side="right") as ptern:
            w1r_ = w1T.rearrange("(kt p) o -> p kt o", p=P)
            for m in range(M_FF):
                wc = ptern.tile([P, M_SEM, P], F32, tag="w1c", bufs=1)
                nc.sync.dma_start(wc[:], w1r_[:, :, m * P:(m + 1) * P])
                tw = ptern.tile([P, M_SEM * P], F32, tag="terntmp", bufs=1)
                wcf = wc[:].rearrange("p a b -> p (a b)")
                nc.vector.tensor_scalar(tw[:], wcf, sw1_bc, None, ALU.mult)
                nc.vector.tensor_scalar(tw[:], tw[:], 1.49, -1.49, ALU.min,
                                        ALU.max)
                w1q = ptern.tile([P, M_SEM, P], BF16, tag="w1q", bufs=1)
                nc.vector.tensor_scalar(w1q[:].rearrange("p a b -> p (a b)"),
                                        tw[:], C_RND, C_RND, ALU.add,
                                        ALU.subtract)
                nc.sync.dma_start(w1q_d[:, m], w1q[:])
            w2r_ = w2T.rearrange("(kt p) o -> p kt o", p=P)
            for m in range(M_SEM):
                for sub in range(4):
                    wc2 = ptern.tile([P, M_SEM, P], F32, tag="w1c", bufs=1)
                    nc.sync.dma_start(
                        wc2[:], w2r_[:, sub * M_SEM:(sub + 1) * M_SEM,
                                     m * P:(m + 1) * P])
                    tw2 = ptern.tile([P, M_SEM * P], F32, tag="terntmp", bufs=1)
                    wcf2 = wc2[:].rearrange("p a b -> p (a b)")
                    nc.vector.tensor_scalar(tw2[:], wcf2, sw2_bc, None, ALU.mult)
                    nc.vector.tensor_scalar(tw2[:], tw2[:], 1.49, -1.49, ALU.min,
                                            ALU.max)
                    w2q2 = ptern.tile([P, M_SEM, P], BF16, tag="w1q", bufs=1)
                    nc.vector.tensor_scalar(
                        w2q2[:].rearrange("p a b -> p (a b)"), tw2[:], C_RND,
                        C_RND, ALU.add, ALU.subtract)
                    nc.sync.dma_start(
                        w2q_d[:, m, sub * M_SEM:(sub + 1) * M_SEM], w2q2[:])


        if debug_outs:
            nc.sync.dma_start(dbg["dbg_q"].rearrange("(m p) t -> p m t", p=P), q_sb[:])
            nc.sync.dma_start(dbg["dbg_k"].rearrange("(m p) t -> p m t", p=P), k_sb[:])
            nc.sync.dma_start(dbg["dbg_v"].rearrange("(m p) t -> p m t", p=P), v_sb[:])

        # ====== phases 4-9: token-half pipeline (overlap via per-half deps) ======
        es_so = ExitStack()
        psem = es_so.enter_context(tc.tile_pool(name="psem", bufs=1))
        semout_n = [psem.tile([P, M_SEM, 512], F32, tag=f"so{n}", name=f"so{n}")
                    for n in range(NT_Q)]
        es_opr = ExitStack()
        popr = es_opr.enter_context(tc.tile_pool(name="popr", bufs=1))
        wo_sb = popr.tile([P, M_SEM, DS], BF16)
        nc.sync.dma_start(wo_sb[:], woT.rearrange("(m p) o -> p m o", p=P))

        es_ctx = ExitStack()
        pctx = es_ctx.enter_context(tc.tile_pool(name="pctx", bufs=1))
        ctx_n = [pctx.tile([P, M_SEM, 512], BF16, tag=f"ctx{n}", name=f"ctx{n}")
                 for n in range(NT_Q)]

        with tc.tile_pool(name="pattn", bufs=1) as pattn, \
             tc.tile_pool(name="ps_s", bufs=5, space="PSUM") as ps_s:
            for n in range(NT_Q):
                for h in range(H):
                    pt = pattn.tile([P, NT_K, 512], BF16, tag="ptile", bufs=2)
                    for mt in range(NT_K):
                        ps = ps_s.tile([P, 512], F32, tag="sps")
                        nc.tensor.matmul(ps[:], k_sb[:, h, mt * P:(mt + 1) * P],
                                         q_sb[:, h, n * 512:(n + 1) * 512],
                                         start=True, stop=True)
                        nc.scalar.activation(pt[:, mt, :], ps[:], AF.Exp,
                                             scale=QK_SCALE)
                    td = pattn.tile([P, 8, 512], BF16, tag="dentree", bufs=1)
                    ptf = pt[:].rearrange("p a b -> p (a b)")
                    tdf = td[:].rearrange("p a b -> p (a b)")
                    nc.vector.tensor_tensor(tdf[:, 0:4096], ptf[:, 0:4096],
                                            ptf[:, 4096:8192], op=ALU.add)
                    nc.vector.tensor_tensor(tdf[:, 0:2048], tdf[:, 0:2048],
                                            tdf[:, 2048:4096], op=ALU.add)
                    nc.vector.tensor_tensor(tdf[:, 0:1024], tdf[:, 0:1024],
                                            tdf[:, 1024:2048], op=ALU.add)
                    nc.vector.tensor_tensor(tdf[:, 0:512], tdf[:, 0:512],
                                            tdf[:, 512:1024], op=ALU.add)
                    den_all = pattn.tile([P, 512], F32, tag="denall", bufs=2)
                    nc.gpsimd.partition_all_reduce(den_all[:], td[:, 0, :], P,
                                                   bass_isa.ReduceOp.add)
                    rden_bc = pattn.tile([P, 512], F32, tag="rdenbc", bufs=2)
                    nc.vector.reciprocal_approx_fast(rden_bc[:], den_all[:])
                    cps = ps_mm.tile([P, 512], F32, tag="mm")
                    for mt in range(NT_K):
                        nc.tensor.matmul(cps[:], v_sb[:, mt, h * P:(h + 1) * P],
                                         pt[:, mt, :],
                                         start=(mt == 0), stop=(mt == NT_K - 1))
                    tnorm = pattn.tile([P, 512], F32, tag="ctxnorm", bufs=2)
                    nc.vector.tensor_tensor(tnorm[:], cps[:], rden_bc[:],
                                            op=ALU.mult)
                    nc.vector.tensor_scalar(ctx_n[n][:, h, :], tnorm[:],
                                            bv_sb[:, h:h + 1], None, ALU.add)
        es_qkv.close()

        # ---- out-proj ----
        if True:
            for n in range(NT_Q):
                for m in range(M_SEM):
                    semres = popr.tile([P, 512], F32, tag="semres", bufs=2)
                    nc.sync.dma_start(semres[:],
                                      semT_r[:, m, n * 512:(n + 1) * 512])
                    ps = ps_mm.tile([P, 512], F32, tag="mm")
                    for kk in range(M_SEM):
                        nc.tensor.matmul(ps[:],
                                         wo_sb[:, kk, m * P:(m + 1) * P],
                                         ctx_n[n][:, kk, :],
                                         start=(kk == 0),
                                         stop=(kk == M_SEM - 1))
                    t = popr.tile([P, 512], F32, tag="oproj", bufs=3)
                    nc.scalar.activation(t[:], ps[:], AF.Identity,
                                         bias=bo_sb[:, m:m + 1])
                    nc.vector.tensor_tensor(semout_n[n][:, m, :], t[:],
                                            semres[:], op=ALU.add)
        es_ctx.close()
        es_opr.close()

        # ---- FFN tensors (right side) ----
        es_h = ExitStack()
        ph = es_h.enter_context(tc.tile_pool(name="ph", bufs=1, side="right"))
        h_n = [ph.tile([P, M_FF, 512], BF16, tag=f"h{n}", name=f"h{n}") for n in range(NT_Q)]
        mx2_n = [ph.tile([P, 512], BF16, tag=f"mx2{n}", name=f"mx2{n}") for n in range(NT_Q)]
        mn2_n = [ph.tile([P, 512], BF16, tag=f"mn2{n}", name=f"mn2{n}") for n in range(NT_Q)]
        shbc_n = [ph.tile([P, 512], F32, tag=f"shbc{n}", name=f"shbc{n}") for n in range(NT_Q)]
        dq2_n = [ph.tile([P, 512], F32, tag=f"dq2{n}", name=f"dq2{n}") for n in range(NT_Q)]

        es_xq = ExitStack()
        pxq = es_xq.enter_context(tc.tile_pool(name="pxq", bufs=1,
                                               side="right"))
        xq_n = [pxq.tile([P, M_SEM, 512], BF16, tag=f"xq{n}", name=f"xq{n}")
                for n in range(NT_Q)]
        sxbc_n = [pxq.tile([P, 512], F32, tag=f"sxbc{n}", name=f"sxbc{n}")
                  for n in range(NT_Q)]
        rsxbc_n = [pxq.tile([P, 512], F32, tag=f"rsxbc{n}", name=f"rsxbc{n}")
                   for n in range(NT_Q)]

        # ---- whole FFN complex in ONE scratch scope (no pool barriers) ----
        with tc.tile_pool(name="pffs", bufs=1) as pffs:
            def ffnorm_xquant(n):
                xn = pffs.tile([P, M_SEM, 512], BF16, tag="xn", bufs=1)
                rmsnorm_fm(pffs, lambda m: semout_n[n][:, m, :], M_SEM, 512,
                           gff_sb, xn)
                mx = pffs.tile([P, 512], BF16, tag="bt", bufs=4)
                mn = pffs.tile([P, 512], BF16, tag="bt", bufs=4)
                nc.vector.tensor_tensor(mx[:], xn[:, 0, :], xn[:, 1, :],
                                        op=ALU.max)
                nc.vector.tensor_tensor(mn[:], xn[:, 0, :], xn[:, 1, :],
                                        op=ALU.min)
                for m in range(2, M_SEM):
                    nc.vector.tensor_tensor(mx[:], mx[:], xn[:, m, :],
                                            op=ALU.max)
                    nc.vector.tensor_tensor(mn[:], mn[:], xn[:, m, :],
                                            op=ALU.min)
                am = pffs.tile([P, 512], BF16, tag="bt", bufs=4)
                nc.vector.scalar_tensor_tensor(out=am[:], in0=mn[:],
                                               scalar=-1.0, in1=mx[:],
                                               op0=ALU.mult, op1=ALU.max)
                amc = pffs.tile([P, 512], F32, tag="ft", bufs=2)
                nc.gpsimd.partition_all_reduce(amc[:], am[:], P,
                                               bass_isa.ReduceOp.absmax)
                nc.vector.tensor_scalar(amc[:], amc[:], 1e-5, None, ALU.max)
                nc.vector.reciprocal_approx_fast(sxbc_n[n][:], amc[:])
                nc.vector.tensor_scalar(sxbc_n[n][:], sxbc_n[n][:], 127.0,
                                        None, ALU.mult)
                nc.vector.tensor_scalar(rsxbc_n[n][:], amc[:], 1.0 / 127.0,
                                        None, ALU.mult)
                tq = pffs.tile([P, M_SEM, 512], F32, tag="qtw", bufs=1)
                tqf = tq[:].rearrange("p a b -> p (a b)")
                nc.vector.tensor_tensor(tq[:], xn[:],
                                        bcast_free(sxbc_n[n][:], M_SEM),
                                        op=ALU.mult)
                nc.vector.tensor_scalar(tqf[:], tqf[:], C_RND, C_RND, ALU.add,
                                        ALU.subtract)
                nc.vector.tensor_tensor(xq_n[n][:], tq[:],
                                        bcast_free(rsxbc_n[n][:], M_SEM),
                                        op=ALU.mult)

            def ffn1(n):
                for m in range(M_FF):
                    w1q = pffs.tile([P, M_FF, P], BF16, tag="wq", bufs=2)
                    nc.sync.dma_start(w1q[:, :M_SEM, :], w1q_d[:, m])
                    ps = ps_mm.tile([P, 512], F32, tag="mm")
                    for kk in range(M_SEM):
                        nc.tensor.matmul(ps[:], w1q[:, kk, :],
                                         xq_n[n][:, kk, :],
                                         start=(kk == 0),
                                         stop=(kk == M_SEM - 1))
                    sn = pffs.tile([P, 512], BF16, tag="bt", bufs=4)
                    nc.scalar.activation(sn[:], ps[:], AF.Sin,
                                         scale=alphap[:, m:m + 1])
                    sq2 = pffs.tile([P, 512], BF16, tag="bt", bufs=4)
                    nc.scalar.activation(sq2[:], sn[:], AF.Square)
                    nc.vector.scalar_tensor_tensor(
                        out=h_n[n][:, m, :], in0=sq2[:],
                        scalar=rbetap[:, m:m + 1], in1=ps[:],
                        op0=ALU.mult, op1=ALU.add)
                    if m == 0:
                        nc.vector.tensor_copy(mx2_n[n][:], h_n[n][:, 0, :])
                        nc.vector.tensor_copy(mn2_n[n][:], h_n[n][:, 0, :])
                    else:
                        nc.vector.tensor_tensor(mx2_n[n][:], mx2_n[n][:],
                                                h_n[n][:, m, :], op=ALU.max)
                        nc.vector.tensor_tensor(mn2_n[n][:], mn2_n[n][:],
                                                h_n[n][:, m, :], op=ALU.min)

            def hquant(n):
                am2 = pffs.tile([P, 512], BF16, tag="bt", bufs=4)
                nc.vector.scalar_tensor_tensor(out=am2[:], in0=mn2_n[n][:],
                                               scalar=-1.0, in1=mx2_n[n][:],
                                               op0=ALU.mult, op1=ALU.max)
                amc2 = pffs.tile([P, 512], F32, tag="ft", bufs=2)
                nc.gpsimd.partition_all_reduce(amc2[:], am2[:], P,
                                               bass_isa.ReduceOp.absmax)
                nc.vector.tensor_scalar(amc2[:], amc2[:], mw1, 1e-5, ALU.mult,
                                        ALU.max)
                nc.vector.reciprocal_approx_fast(shbc_n[n][:], amc2[:])
                nc.vector.tensor_scalar(shbc_n[n][:], shbc_n[n][:], mw1, 127.0,
                                        ALU.mult, ALU.mult)
                nc.vector.tensor_scalar(dq2_n[n][:], amc2[:], mw2, 1.0 / 127.0,
                                        ALU.mult, ALU.mult)
                for c4 in range(M_FF // M_SEM):
                    tq2 = pffs.tile([P, M_SEM, 512], F32, tag="qtw", bufs=1)
                    tq2f = tq2[:].rearrange("p a b -> p (a b)")
                    nc.vector.tensor_tensor(
                        tq2[:], h_n[n][:, c4 * M_SEM:(c4 + 1) * M_SEM, :],
                        bcast_free(shbc_n[n][:], M_SEM), op=ALU.mult)
                    nc.vector.tensor_scalar(
                        h_n[n][:, c4 * M_SEM:(c4 + 1) * M_SEM, :]
                        .rearrange("p a b -> p (a b)"),
                        tq2f[:], C_RND, C_RND, ALU.add, ALU.subtract)

            def ffn2(n):
                for m in range(M_SEM):
                    w2q = pffs.tile([P, M_FF, P], BF16, tag="wq", bufs=2)
                    nc.sync.dma_start(w2q[:], w2q_d[:, m])
                    ps = ps_mm.tile([P, 512], F32, tag="mm")
                    for kk in range(M_FF):
                        nc.tensor.matmul(ps[:], w2q[:, kk, :], h_n[n][:, kk, :],
                                         start=(kk == 0),
                                         stop=(kk == M_FF - 1))
                    t = pffs.tile([P, 512], F32, tag="qt", bufs=3)
                    nc.vector.tensor_tensor(t[:], ps[:], dq2_n[n][:],
                                            op=ALU.mult)
                    yo = pffs.tile([P, 512], F32, tag="qt", bufs=3)
                    nc.vector.tensor_tensor(yo[:], t[:], semout_n[n][:, m, :],
                                            op=ALU.add)
                    nc.sync.dma_start(outT[m * P:(m + 1) * P,
                                           n * 512:(n + 1) * 512], yo[:])

            ffnorm_xquant(0)
            ffnorm_xquant(1)
            ffn1(0)
            ffn1(1)
            hquant(0)
            hquant(1)
            ffn2(0)
            ffn2(1)
        es_xq.close()
        es_h.close()
        es_so.close()

    nc.compile()
    return nc


_NC_CACHE = {}


def _get_nc(debug_outs=False):
    key = bool(debug_outs)
    if key not in _NC_CACHE:
        _NC_CACHE[key] = build_nc(debug_outs)
    return _NC_CACHE[key]


def make_in_maps(inputs):
    """Host-side shard + layout prep. inputs: dict of full np arrays."""
    import ml_dtypes
    bf = ml_dtypes.bfloat16
    f32 = np.float32
    sem = np.asarray(inputs["sem"], f32)
    pro = np.asarray(inputs["pro"], f32)

    def cols(v, nm):
        return np.ascontiguousarray(np.asarray(v, f32).reshape(nm, P).T)

    common = {
        "gsem": cols(inputs["g_sem"], M_SEM),
        "gpro": cols(inputs["g_pro"], M_PRO),
        "gff": cols(inputs["g_ff"], M_SEM),
        "bq": cols(inputs["bq"], M_SEM),
        "bk": cols(inputs["bk"], M_SEM),
        "bv": cols(inputs["bv"], M_SEM),
        "bo": cols(inputs["bo"], M_SEM),
        "alpha": cols(inputs["alpha"], M_FF),
        "beta": cols(inputs["beta"], M_FF),
        "w1T": np.ascontiguousarray(np.asarray(inputs["W1"], f32).T),
        "w2T": np.ascontiguousarray(np.asarray(inputs["W2"], f32).T),
        "wqT": np.ascontiguousarray(np.asarray(inputs["Wq"], f32).T).astype(bf),
        "wkT": np.ascontiguousarray(np.asarray(inputs["Wk"], f32).T).astype(bf),
        "wvT": np.ascontiguousarray(np.asarray(inputs["Wv"], f32).T).astype(bf),
        "woT": np.ascontiguousarray(np.asarray(inputs["Wo"], f32).T).astype(bf),
    }

    in_maps = []
    for c in range(N_CORES):
        b, half = c // 2, c % 2
        m = dict(common)
        m["semT"] = np.ascontiguousarray(sem[b, half * TOK:(half + 1) * TOK, :].T)
        m["proT"] = np.ascontiguousarray(pro[b].T)
        m["w1s"] = np.ascontiguousarray(common["w1T"][c * P:(c + 1) * P, :])
        m["w2s"] = np.ascontiguousarray(common["w2T"][c * DP:(c + 1) * DP, :])
        in_maps.append(m)
    return in_maps


def assemble_out(results):
    out = np.empty((B, S, DS), np.float32)
    for c in range(N_CORES):
        b, half = c // 2, c % 2
        out[b, half * TOK:(half + 1) * TOK, :] = results[c]["outT"].T
    return out


def kernel(**inputs):
    nc = _get_nc()
    in_maps = make_in_maps(inputs)
    res = run_bass_kernel_spmd(nc, in_maps, core_ids=list(range(N_CORES)))
    return assemble_out(res.results)



# revision 22
# speedup vs baseline: 1.7572x; 1.7572x over previous
"""Trainium2 Bass kernel for nn_CrossAttentionFusion (cross-attention + BitLinear FFN).

Sharding: 8 cores = 4 batches x 2 sequence-halves (data parallel, no collectives).
Each core owns 1024 query tokens; computes K/V for its batch's full 2048 tokens.

v2: fp8 DoubleRow matmuls everywhere (QKV/O/PV/FFN), host-side ternarization of
W1/W2 (shipped as fp8 +-1/0), softmax denominator computed on the PE via a
ones-matmul, act_quant realized as a direct fp8 cast.
"""
import math
import numpy as np
from contextlib import ExitStack

import concourse.bass as bass
import concourse.bass_isa as bass_isa
import concourse.tile as tile
from concourse import bacc, mybir
from concourse.bass_utils import run_bass_kernel_spmd

F32 = mybir.dt.float32
F32R = mybir.dt.float32r
BF16 = mybir.dt.bfloat16
FP8 = mybir.dt.float8e4
AF = mybir.ActivationFunctionType
ALU = mybir.AluOpType
DR = mybir.MatmulPerfMode.DoubleRow

B, S, DS, DP, H = 4, 2048, 1024, 512, 8
DF = 4 * DS
HD = DS // H          # 128
TOK = 1024            # query tokens per core
N_CORES = 8
EPS = 1e-6
QK_SCALE = 1.0 / math.sqrt(HD)
HALF_PI = math.pi / 2.0
WS = 32.0             # fp8 scale for Wq/Wk/Wv/Wo
CTX_S = 64.0          # fp8 scale for ctx (folded into 1/den via ones=1/64)

P = 128
M_SEM = DS // P       # 8
M_PRO = DP // P       # 4
M_FF = DF // P        # 32
NT_K = S // P         # 16


def build_nc(debug_outs=False):
    nc = bacc.Bacc("TRN2", target_bir_lowering=False, debug=False, num_devices=N_CORES)

    semT = nc.dram_tensor("semT", [DS, TOK], F32, kind="ExternalInput").ap()
    proT = nc.dram_tensor("proT", [DP, S], BF16, kind="ExternalInput").ap()
    wq = nc.dram_tensor("wq", [P, M_SEM, M_SEM, P], FP8, kind="ExternalInput").ap()
    wk = nc.dram_tensor("wk", [P, M_PRO, M_SEM, P], FP8, kind="ExternalInput").ap()
    wv = nc.dram_tensor("wv", [P, M_PRO, DS], FP8, kind="ExternalInput").ap()
    wo = nc.dram_tensor("wo", [P, M_SEM, M_SEM, P], FP8, kind="ExternalInput").ap()
    w1q = nc.dram_tensor("w1q", [P, M_SEM, M_FF, P], FP8, kind="ExternalInput").ap()
    w2q = nc.dram_tensor("w2q", [P, M_FF, M_SEM, P], FP8, kind="ExternalInput").ap()
    gsem = nc.dram_tensor("gsem", [P, M_SEM], F32, kind="ExternalInput").ap()
    gpro = nc.dram_tensor("gpro", [P, M_PRO], F32, kind="ExternalInput").ap()
    gff = nc.dram_tensor("gff", [P, M_SEM], F32, kind="ExternalInput").ap()
    bq = nc.dram_tensor("bq", [P, M_SEM], F32, kind="ExternalInput").ap()
    bk = nc.dram_tensor("bk", [P, M_SEM], F32, kind="ExternalInput").ap()
    obias = nc.dram_tensor("obias", [P, M_SEM], F32, kind="ExternalInput").ap()
    ybias = nc.dram_tensor("ybias", [P, M_SEM], F32, kind="ExternalInput").ap()
    acos = nc.dram_tensor("acos", [P, M_FF], F32, kind="ExternalInput").ap()
    nrb2 = nc.dram_tensor("nrb2", [P, M_FF], F32, kind="ExternalInput").ap()
    dqs = nc.dram_tensor("dqs", [P, 1], F32, kind="ExternalInput").ap()
    outT = nc.dram_tensor("outT", [DS, TOK], F32, kind="ExternalOutput").ap()

    semT_r = semT.rearrange("(m p) t -> p m t", p=P)
    proT_r = proT.rearrange("(m p) t -> p m t", p=P)
    outT_r = outT.rearrange("(m p) t -> p m t", p=P)

    with tile.TileContext(nc) as tc, ExitStack() as top:
        persist = top.enter_context(tc.tile_pool(name="persist", bufs=1))

        ones = persist.tile([P, 1], BF16)
        nc.vector.memset(ones[:], 1.0)
        ones_row = persist.tile([1, P], BF16)
        nc.vector.memset(ones_row[:], 1.0)
        ones2_f8 = persist.tile([P, 2, 16], FP8)
        nc.vector.memset(ones2_f8[:], 1.0 / CTX_S)
        eps_t = persist.tile([1, 1], F32)
        nc.vector.memset(eps_t[:], EPS)
        hpi_t = persist.tile([P, 1], F32)
        nc.vector.memset(hpi_t[:], HALF_PI)

        gsem_sb = persist.tile([P, M_SEM], F32)
        gpro_sb = persist.tile([P, M_PRO], F32)
        gff_sb = persist.tile([P, M_SEM], F32)
        bq_sb = persist.tile([P, M_SEM], F32)
        bk_sb = persist.tile([P, M_SEM], F32)
        obias_sb = persist.tile([P, M_SEM], F32)
        ybias_sb = persist.tile([P, M_SEM], F32)
        acos_sb = persist.tile([P, M_FF], F32)
        nrb2_sb = persist.tile([P, M_FF], F32)
        dqs_sb = persist.tile([P, 1], F32)
        for ap_d, t in [(gsem, gsem_sb), (gpro, gpro_sb), (gff, gff_sb),
                        (bq, bq_sb), (bk, bk_sb), (obias, obias_sb),
                        (ybias, ybias_sb), (acos, acos_sb), (nrb2, nrb2_sb),
                        (dqs, dqs_sb)]:
            nc.sync.dma_start(t[:], ap_d[:])

        # persistent big tensors
        semT_sb = persist.tile([P, M_SEM, TOK], F32)      # becomes semout in place
        es_att = ExitStack()
        patt_io = es_att.enter_context(tc.tile_pool(name="patt_io", bufs=1))
        q_sb = patt_io.tile([P, M_SEM, TOK], FP8)
        k_sb = patt_io.tile([P, M_SEM, S], FP8)
        v_sb = patt_io.tile([P, NT_K, DS], FP8)
        ctx_n = [persist.tile([P, M_SEM, 512], FP8, name=f"ctx{n}")
                 for n in range(2)]

        for m2 in range(4):
            nc.sync.dma_start(semT_sb[:, 2 * m2:2 * m2 + 2, :],
                              semT_r[:, 2 * m2:2 * m2 + 2, :])

        # ---------- input norms -> semn/pron in fp8 ----------
        es_norm = ExitStack()
        pnorm = es_norm.enter_context(tc.tile_pool(name="pnorm", bufs=1))
        semn = pnorm.tile([P, M_SEM, TOK], FP8)
        pron = pnorm.tile([P, M_PRO, S], FP8)
        ps_nrm = es_norm.enter_context(
            tc.tile_pool(name="ps_nrm", bufs=2, space="PSUM"))

        with tc.tile_pool(name="pnsc", bufs=1) as pnsc:
            proT_sb = pnsc.tile([P, M_PRO, S], BF16)
            for m in range(M_PRO):
                nc.sync.dma_start(proT_sb[:, m, :], proT_r[:, m, :])

            def rmsnorm(x_sb, nm, T, g, out_f8):
                D = nm * P
                sq = pnsc.tile([P, nm * T], BF16, tag="sq", bufs=1, name="sq")
                xf = x_sb[:].rearrange("p a b -> p (a b)")
                half = nm * T // 2
                nc.scalar.activation(sq[:, :half], xf[:, :half], AF.Square)
                nc.scalar.activation(sq[:, half:], xf[:, half:], AF.Square)
                rs_row = pnsc.tile([1, T], F32, tag=f"rs{nm}", bufs=1)
                for ch in range(T // 512):
                    pst = ps_nrm.tile([P, 512], F32, tag="nrm")
                    ps = pst[0:1, :]
                    for m in range(nm):
                        nc.tensor.matmul(
                            ps[:], ones[:],
                            sq[:, m * T + ch * 512:m * T + (ch + 1) * 512],
                            start=(m == 0), stop=(m == nm - 1))
                    nc.scalar.activation(rs_row[:, ch * 512:(ch + 1) * 512],
                                         ps[:], AF.Ln, bias=eps_t[:],
                                         scale=1.0 / D)
                nc.scalar.activation(rs_row[:], rs_row[:], AF.Exp, scale=-0.5)
                rs_bc = pnsc.tile([P, T], F32, tag=f"rsbc{nm}", bufs=1)
                nc.gpsimd.partition_broadcast(rs_bc[:], rs_row[:])
                for m in range(nm):
                    nc.vector.scalar_tensor_tensor(
                        out=out_f8[:, m, :], in0=x_sb[:, m, :],
                        scalar=g[:, m:m + 1], in1=rs_bc[:],
                        op0=ALU.mult, op1=ALU.mult)

            rmsnorm(semT_sb, M_SEM, TOK, gsem_sb, semn)
            rmsnorm(proT_sb, M_PRO, S, gpro_sb, pron)

        # ---------- QKV projections (fp8 DoubleRow) ----------
        es_qw = ExitStack()
        pqw = es_qw.enter_context(tc.tile_pool(name="pqw", bufs=1, side="right"))
        wq_sb = pqw.tile([P, M_SEM, M_SEM, P], FP8)
        wk_sb = pqw.tile([P, M_PRO, M_SEM, P], FP8)
        wv_sb = pqw.tile([P, M_PRO, DS], FP8)
        nc.sync.dma_start(wq_sb[:], wq[:])
        nc.sync.dma_start(wk_sb[:], wk[:])
        nc.sync.dma_start(wv_sb[:], wv[:])

        with tc.tile_pool(name="ps_mm", bufs=2, space="PSUM") as ps_mm:
            # Q: out q_sb[:, m, 0:1024]
            for m in range(M_SEM):
                ps = ps_mm.tile([P, 1024], F32, tag="mm")
                for n2 in range(2):
                    for kk in range(4):
                        nc.tensor.matmul(
                            ps[:, n2 * 512:(n2 + 1) * 512],
                            wq_sb[:, 2 * kk:2 * kk + 2, m, :],
                            semn[:, 2 * kk:2 * kk + 2, n2 * 512:(n2 + 1) * 512],
                            start=(kk == 0), stop=(kk == 3), perf_mode=DR)
                nc.vector.tensor_scalar(q_sb[:, m, :], ps[:], 1.0 / WS,
                                        bq_sb[:, m:m + 1], ALU.mult, ALU.add)
            # K: out k_sb[:, m, 0:2048]
            for m in range(M_SEM):
                for c2 in range(2):
                    ps = ps_mm.tile([P, 1024], F32, tag="mm")
                    for n2 in range(2):
                        col = (c2 * 2 + n2) * 512
                        for kk in range(2):
                            nc.tensor.matmul(
                                ps[:, n2 * 512:(n2 + 1) * 512],
                                wk_sb[:, 2 * kk:2 * kk + 2, m, :],
                                pron[:, 2 * kk:2 * kk + 2, col:col + 512],
                                start=(kk == 0), stop=(kk == 1), perf_mode=DR)
                    nc.scalar.activation(k_sb[:, m, c2 * 1024:(c2 + 1) * 1024],
                                         ps[:], AF.Identity, scale=1.0 / WS,
                                         bias=bk_sb[:, m:m + 1])
            # V (transposed, kpos-major): out v_sb[:, mt, 0:1024]
            for mt in range(NT_K):
                ps = ps_mm.tile([P, 1024], F32, tag="mm")
                for n2 in range(2):
                    for kk in range(2):
                        nc.tensor.matmul(
                            ps[:, n2 * 512:(n2 + 1) * 512],
                            pron[:, 2 * kk:2 * kk + 2, mt * P:(mt + 1) * P],
                            wv_sb[:, 2 * kk:2 * kk + 2, n2 * 512:(n2 + 1) * 512],
                            start=(kk == 0), stop=(kk == 1), perf_mode=DR)
                nc.vector.tensor_scalar(v_sb[:, mt, :], ps[:], 1.0 / WS, None,
                                        ALU.mult)
        es_norm.close()
        es_qw.close()

        # FFN weights arrive during attention
        es_fw = ExitStack()
        pfw = es_fw.enter_context(tc.tile_pool(name="pfw", bufs=1, side="right"))
        w1q_sb = pfw.tile([P, M_SEM, M_FF, P], FP8)
        w2q_sb = pfw.tile([P, M_FF, M_SEM, P], FP8)
        for c4 in range(4):
            nc.sync.dma_start(w1q_sb[:, 2 * c4:2 * c4 + 2], w1q[:, 2 * c4:2 * c4 + 2])
        for c4 in range(4):
            nc.sync.dma_start(w2q_sb[:, 8 * c4:8 * c4 + 8], w2q[:, 8 * c4:8 * c4 + 8])
        wo_sb = pfw.tile([P, M_SEM, M_SEM, P], FP8)
        nc.sync.dma_start(wo_sb[:], wo[:])

        # ---------- attention ----------
        with tc.tile_pool(name="pattn", bufs=1) as pattn, \
             tc.tile_pool(name="ps_att", bufs=1, space="PSUM") as ps_att:
            for n in range(2):
                for h in range(H):
                    pt = pattn.tile([P, NT_K, 512], FP8, tag="pt", bufs=2)
                    den = ps_att.tile([P, 512], F32, tag="den", bufs=1)
                    cps = ps_att.tile([P, 512], F32, tag="pv", bufs=1)
                    for kp in range(8):
                        sc = ps_att.tile([P, 1024], F32, tag="sc", bufs=2)
                        nc.tensor.matmul(
                            sc[:, 0:512],
                            k_sb[:, h, (2 * kp) * P:(2 * kp + 1) * P],
                            q_sb[:, h, n * 512:(n + 1) * 512],
                            start=True, stop=True)
                        nc.tensor.matmul(
                            sc[:, 512:1024],
                            k_sb[:, h, (2 * kp + 1) * P:(2 * kp + 2) * P],
                            q_sb[:, h, n * 512:(n + 1) * 512],
                            start=True, stop=True)
                        nc.scalar.activation(
                            pt[:, 2 * kp:2 * kp + 2, :].rearrange(
                                "p a b -> p (a b)"),
                            sc[:], AF.Exp, scale=QK_SCALE)
                        nc.tensor.matmul(den[0:1, :], ones2_f8[:, :, 0:1],
                                         pt[:, 2 * kp:2 * kp + 2, :],
                                         start=(kp == 0), stop=(kp == 7),
                                         perf_mode=DR)
                        nc.tensor.matmul(cps[:],
                                         v_sb[:, 2 * kp:2 * kp + 2,
                                              h * P:(h + 1) * P],
                                         pt[:, 2 * kp:2 * kp + 2, :],
                                         start=(kp == 0), stop=(kp == 7),
                                         perf_mode=DR)
                    rden = pattn.tile([1, 512], F32, tag="rden", bufs=2)
                    nc.vector.reciprocal_approx_fast(rden[:], den[0:1, :])
                    rdenb = pattn.tile([1, 512], BF16, tag="rdenb", bufs=2)
                    nc.gpsimd.tensor_copy(rdenb[:], rden[:])
                    bc = ps_att.tile([P, 512], F32, tag="bc", bufs=2)
                    nc.tensor.matmul(bc[:], ones_row[:], rdenb[:],
                                     start=True, stop=True)
                    bc_sb = pattn.tile([P, 512], BF16, tag="bcsb", bufs=2)
                    nc.vector.tensor_copy(bc_sb[:], bc[:])
                    nc.vector.tensor_tensor(ctx_n[n][:, h, :], cps[:], bc_sb[:],
                                            op=ALU.mult)
        es_att.close()

        # ---------- out-proj + ffn-norm + FFN (per half) ----------
        with tc.tile_pool(name="pff", bufs=1) as pff, \
             tc.tile_pool(name="ps_ff", bufs=2, space="PSUM") as ps_ff:
            xq_n = [pff.tile([P, M_SEM, 512], FP8, name=f"xq{n}")
                    for n in range(2)]
            h2_n = [pff.tile([P, M_FF, 512], FP8, name=f"h2{n}")
                    for n in range(2)]
            for n in range(2):
                ncol = slice(n * 512, (n + 1) * 512)
                # out-proj
                t_o = pff.tile([P, M_SEM, 512], BF16, tag="t_o", bufs=1)
                for m in range(M_SEM):
                    ps = ps_ff.tile([P, 512], F32, tag="mmf")
                    for kk in range(4):
                        nc.tensor.matmul(ps[:], wo_sb[:, 2 * kk:2 * kk + 2, m, :],
                                         ctx_n[n][:, 2 * kk:2 * kk + 2, :],
                                         start=(kk == 0), stop=(kk == 3),
                                         perf_mode=DR)
                    nc.vector.tensor_scalar(t_o[:, m, :], ps[:],
                                            1.0 / (WS * CTX_S),
                                            obias_sb[:, m:m + 1],
                                            ALU.mult, ALU.add)
                semo = semT_sb[:, :, ncol]
                nc.vector.tensor_tensor(semo, t_o[:], semo, op=ALU.add)

                # ffn norm -> xq (fp8)
                sqf = pff.tile([P, M_SEM, 512], BF16, tag="sqf", bufs=1)
                nc.gpsimd.tensor_tensor(sqf[:], semo, semo, op=ALU.mult)
                pst = ps_ff.tile([P, 512], F32, tag="nrmf")
                psr = pst[0:1, :]
                for m in range(M_SEM):
                    nc.tensor.matmul(psr[:], ones[:], sqf[:, m, :],
                                     start=(m == 0), stop=(m == M_SEM - 1))
                rsf = pff.tile([1, 512], F32, tag="rsf", bufs=1)
                nc.scalar.activation(rsf[:], psr[:], AF.Ln, bias=eps_t[:],
                                     scale=1.0 / DS)
                nc.scalar.activation(rsf[:], rsf[:], AF.Exp, scale=-0.5)
                rsbc = pff.tile([P, 512], F32, tag="rsbc", bufs=1)
                nc.gpsimd.partition_broadcast(rsbc[:], rsf[:])
                for m in range(M_SEM):
                    nc.vector.scalar_tensor_tensor(
                        out=xq_n[n][:, m, :], in0=semT_sb[:, m, ncol],
                        scalar=gff_sb[:, m:m + 1], in1=rsbc[:],
                        op0=ALU.mult, op1=ALU.mult)

                # FFN1 + snake
                for m in range(M_FF):
                    ps = ps_ff.tile([P, 512], F32, tag="mmf")
                    for kk in range(4):
                        nc.tensor.matmul(ps[:], w1q_sb[:, 2 * kk:2 * kk + 2, m, :],
                                         xq_n[n][:, 2 * kk:2 * kk + 2, :],
                                         start=(kk == 0), stop=(kk == 3),
                                         perf_mode=DR)
                    sn = pff.tile([P, 512], BF16, tag="sn", bufs=3)
                    nc.scalar.activation(sn[:], ps[:], AF.Sin, bias=hpi_t[:],
                                         scale=acos_sb[:, m:m + 1])
                    nc.vector.scalar_tensor_tensor(
                        out=h2_n[n][:, m, :], in0=sn[:],
                        scalar=nrb2_sb[:, m:m + 1], in1=ps[:],
                        op0=ALU.mult, op1=ALU.add)

            # FFN2 (both halves; h2 ready per half in order)
            for n in range(2):
                ncol = slice(n * 512, (n + 1) * 512)
                yf = pff.tile([P, M_SEM, 512], BF16, tag="yf", bufs=1)
                for m in range(M_SEM):
                    ps = ps_ff.tile([P, 512], F32, tag="mmf")
                    for kk in range(16):
                        nc.tensor.matmul(ps[:], w2q_sb[:, 2 * kk:2 * kk + 2, m, :],
                                         h2_n[n][:, 2 * kk:2 * kk + 2, :],
                                         start=(kk == 0), stop=(kk == 15),
                                         perf_mode=DR)
                    nc.vector.tensor_scalar(yf[:, m, :], ps[:], dqs_sb[:],
                                            ybias_sb[:, m:m + 1],
                                            ALU.mult, ALU.add)
                yo = pff.tile([P, M_SEM, 512], F32, tag="yo", bufs=1)
                nc.vector.tensor_tensor(yo[:], yf[:], semT_sb[:, :, ncol],
                                        op=ALU.add)
                for m2 in range(2):
                    nc.sync.dma_start(
                        outT_r[:, 4 * m2:4 * m2 + 4, ncol],
                        yo[:, 4 * m2:4 * m2 + 4, :])
        es_fw.close()

    nc.compile()
    return nc


_NC_CACHE = {}


def _get_nc(debug_outs=False):
    key = bool(debug_outs)
    if key not in _NC_CACHE:
        _NC_CACHE[key] = build_nc(debug_outs)
    return _NC_CACHE[key]


def make_in_maps(inputs):
    """Host-side shard + layout prep. inputs: dict of full np arrays."""
    import ml_dtypes
    f8 = ml_dtypes.float8_e4m3
    f32 = np.float32
    sem = np.asarray(inputs["sem"], f32)
    pro = np.asarray(inputs["pro"], f32)

    def cols(v, nm):
        return np.ascontiguousarray(np.asarray(v, f32).reshape(nm, P).T)

    def wlay(wT, nkt, nm):
        # [in=nkt*128, out=nm*128] -> [128p, nkt, nm, 128c]
        return np.ascontiguousarray(
            wT.reshape(nkt, P, nm, P).transpose(1, 0, 2, 3))

    WqT = np.asarray(inputs["Wq"], f32).T * WS
    WkT = np.asarray(inputs["Wk"], f32).T * WS
    WvT = np.asarray(inputs["Wv"], f32).T * WS
    WoT = np.asarray(inputs["Wo"], f32).T * WS

    W1 = np.asarray(inputs["W1"], f32)
    W2 = np.asarray(inputs["W2"], f32)
    mw1 = np.maximum(np.abs(W1).mean(), 1e-5)
    mw2 = np.maximum(np.abs(W2).mean(), 1e-5)
    w1s = np.clip(np.round(W1 / mw1), -1, 1)          # [DF, DS]
    w2s = np.clip(np.round(W2 / mw2), -1, 1)          # [DS, DF]

    alpha = np.asarray(inputs["alpha"], f32)
    beta = np.asarray(inputs["beta"], f32)
    acos_v = 2.0 * alpha * mw1
    rb2 = 1.0 / (2.0 * (beta + 1e-9) * mw1)           # positive snake const
    ybias_v = (mw1 * mw2) * (w2s @ rb2)               # [DS]
    obias_v = np.asarray(inputs["bo"], f32) + \
        np.asarray(inputs["Wo"], f32) @ np.asarray(inputs["bv"], f32)

    common = {
        "gsem": cols(inputs["g_sem"], M_SEM),
        "gpro": cols(inputs["g_pro"], M_PRO),
        "gff": cols(inputs["g_ff"], M_SEM),
        "bq": cols(inputs["bq"], M_SEM),
        "bk": cols(inputs["bk"], M_SEM),
        "obias": cols(obias_v, M_SEM),
        "ybias": cols(ybias_v, M_SEM),
        "acos": cols(acos_v, M_FF),
        "nrb2": cols(-rb2, M_FF),
        "dqs": np.full((P, 1), mw1 * mw2, f32),
        "wq": wlay(WqT, M_SEM, M_SEM).astype(f8),
        "wk": wlay(WkT, M_PRO, M_SEM).astype(f8),
        "wv": np.ascontiguousarray(
            WvT.reshape(M_PRO, P, DS).transpose(1, 0, 2)).astype(f8),
        "wo": wlay(WoT, M_SEM, M_SEM).astype(f8),
        "w1q": wlay(np.ascontiguousarray(w1s.T), M_SEM, M_FF).astype(f8),
        "w2q": wlay(np.ascontiguousarray(w2s.T), M_FF, M_SEM).astype(f8),
    }

    in_maps = []
    for c in range(N_CORES):
        b, half = c // 2, c % 2
        m = dict(common)
        m["semT"] = np.ascontiguousarray(sem[b, half * TOK:(half + 1) * TOK, :].T)
        m["proT"] = np.ascontiguousarray(pro[b].T).astype(ml_dtypes.bfloat16)
        in_maps.append(m)
    return in_maps


def assemble_out(results):
    out = np.empty((B, S, DS), np.float32)
    for c in range(N_CORES):
        b, half = c // 2, c % 2
        out[b, half * TOK:(half + 1) * TOK, :] = results[c]["outT"].T
    return out


def kernel(**inputs):
    nc = _get_nc()
    in_maps = make_in_maps(inputs)
    res = run_bass_kernel_spmd(nc, in_maps, core_ids=list(range(N_CORES)))
    return assemble_out(res.results)
